# revision 19
# baseline (speedup 1.0000x reference)
"""Trainium2 Bass kernel for batched global mean pooling (segment mean).

Computes, for N sorted nodes with 64 features and G graphs:
    out[g, f] = mean over nodes n with batch[n] == g of node_features[n, f]
(empty graphs -> zeros), distributed over 8 NeuronCores.

Strategy (graph sharding; no collectives):
  - Core k owns graphs [128k, 128(k+1)). batch is sorted, so each graph's
    nodes are a contiguous row range of node_features.
  - Host (inside kernel(), per call) lays out each core's nodes on a
    [128, T] grid: partition p gets only the nodes of local graph p,
    zero-padded to T = max graph size.
  - Features are split into fp16 (hi, lo) pairs so the PE runs at full
    rate (fp32 matmul is 4x slower); hi + lo recovers fp32 precision
    since the products accumulate into fp32 PSUM.
  - Device: each matmul is identity128.T @ slab for a [128, 7*64] fp16
    slab, accumulating into one [128, 448] f32 PSUM bank: partition =
    local graph. After all chunks: fold the 7 column blocks, multiply by
    host-provided 1/max(count, 1), DMA the [128, 64] result out.
  - Host concatenates the 8 per-core [128, 64] outputs.

The Bass program is compiled per call with the chunk count derived from
the actual input, so any node/graph distribution is handled.
"""

import math

import numpy as np

import concourse.mybir as mybir
import concourse.tile as tile
from concourse import bacc
from concourse.bass_utils import run_bass_kernel_spmd
from concourse.masks import make_identity

NCORES = 8
P = 128  # partitions = local graphs per core
F = 64  # features
B = 7  # tiles (node-rows) per matmul: N = 7*64 = 448 <= 512 f32 PSUM bank
TB = 63  # tiles per full DMA chunk (~1.03 MB per chunk)

# set by tests to capture a profile; harness path leaves these alone
TRACE = False
LAST_RESULTS = None


def _chunks(t_cap):
    """Split t_cap tiles into DMA chunks: full 63-tile chunks, then taper the
    last ~2 chunks to 21 tiles so the PE tail after the final DMA is short."""
    out = []
    t = 0
    taper_zone = 2 * TB if t_cap > 4 * TB else 0
    while t < t_cap:
        n = min(TB if t_cap - t > taper_zone else 3 * B, t_cap - t)
        out.append((t, n))
        t += n
    return out


def _build(t_cap):
    nc = bacc.Bacc("TRN2", target_bir_lowering=False, debug=False, num_devices=NCORES)
    hi = nc.dram_tensor("hi", [P, t_cap * F], mybir.dt.float16, kind="ExternalInput").ap()
    lo = nc.dram_tensor("lo", [P, t_cap * F], mybir.dt.float16, kind="ExternalInput").ap()
    inv = nc.dram_tensor("inv", [P, 1], mybir.dt.float32, kind="ExternalInput").ap()
    out = nc.dram_tensor("out", [P, F], mybir.dt.float32, kind="ExternalOutput").ap()

    chunks = _chunks(t_cap)
    n_mm = 2 * (t_cap // B)
    with tile.TileContext(nc) as tc:
        with (
            tc.tile_pool(name="consts", bufs=1) as consts,
            tc.tile_pool(name="io", bufs=6) as io,
            tc.tile_pool(name="ep", bufs=1) as ep,
            tc.tile_pool(name="acc", bufs=1, space="PSUM") as accp,
        ):
            # build the identity on-device (GpSimd) so the weight preload has
            # no DMA dependency -- an identity DMA would queue behind the
            # first big chunk DMAs and stall the PE ~14 us at kernel start
            ident_sb = consts.tile([P, P], mybir.dt.float16)
            make_identity(nc, ident_sb[:])

            # load the identity into the PE array once; every matmul below
            # reuses it (ldweights=False) instead of reloading 128 columns
            # per matmul (~100 ns each, ~60 us of PE time at ~600 matmuls)
            ldw = nc.tensor.ldweights(ident_sb[:])

            psum = accp.tile([P, B * F], mybir.dt.float32)
            mm = 0
            for t0, nt in chunks:
                hi_t = io.tile([P, TB * F], mybir.dt.float16, tag="hi")
                nc.sync.dma_start(hi_t[:, : nt * F], hi[:, t0 * F : (t0 + nt) * F])
                lo_t = io.tile([P, TB * F], mybir.dt.float16, tag="lo")
                # second HWDGE ring (ACT engine) so hi/lo issue in parallel
                nc.scalar.dma_start(lo_t[:, : nt * F], lo[:, t0 * F : (t0 + nt) * F])
                for t in (hi_t, lo_t):
                    for b in range(nt // B):
                        inst = nc.tensor.matmul(
                            psum[:],
                            ident_sb[:],
                            t[:, b * B * F : (b + 1) * B * F],
                            start=(mm == 0),
                            stop=(mm == n_mm - 1),
                        )
                        inst.ins.ldweights = False
                        if mm == 0:
                            tile.add_dep_helper(
                                inst.ins,
                                ldw.ins,
                                sync=False,
                                reason="identity weights preloaded once",
                            )
                        mm += 1
            assert mm == n_mm

            # emitted after the chunk loop so this tiny transfer doesn't
            # head-of-line block the first chunk on the sync ring
            inv_sb = consts.tile([P, 1], mybir.dt.float32)
            nc.sync.dma_start(inv_sb[:], inv[:])

            # fold the B column blocks: s = sum_b psum[:, b*64:(b+1)*64]
            # (DVE may read at most one PSUM operand per instruction)
            s = ep.tile([P, F], mybir.dt.float32)
            nc.vector.tensor_copy(s[:], psum[:, 0:F])
            for b in range(1, B):
                nc.vector.tensor_add(s[:], s[:], psum[:, b * F : (b + 1) * F])

            res = ep.tile([P, F], mybir.dt.float32)
            nc.vector.tensor_scalar_mul(res[:], s[:], inv_sb[:])
            nc.sync.dma_start(out[:], res[:])

    nc.compile()
    # bacc materializes one Ldweights per Matmult even with ldweights=False;
    # they all reload the same identity (~100 ns of PE time each). Drop the
    # redundant ones — keep any that carry semaphore waits/updates (those
    # park sync state), including the explicit preload which waits on the
    # identity build.
    for fn in nc.m.functions:
        for blk in fn.blocks:
            keep = [
                inst
                for inst in blk.instructions
                if not (
                    isinstance(inst, mybir.InstLdweights)
                    and (
                        inst.sync_info is None
                        or (
                            len(inst.sync_info.on_wait) == 0
                            and len(inst.sync_info.on_update) == 0
                        )
                    )
                )
            ]
            if len(keep) != len(blk.instructions):
                blk.instructions = keep
    return nc


def kernel(node_features, batch, num_graphs):
    global LAST_RESULTS
    x = np.asarray(node_features, dtype=np.float32)
    b = np.asarray(batch, dtype=np.int64).ravel()
    G = int(num_graphs)
    N = x.shape[0]
    assert x.shape[1] == F, f"expected {F} features, got {x.shape[1]}"

    if not np.all(b[1:] >= b[:-1]):  # defensive: layout relies on sorted batch
        order = np.argsort(b, kind="stable")
        b = b[order]
        x = x[order]

    gpc = math.ceil(G / NCORES)  # local graphs per core
    assert gpc <= P, f"num_graphs {G} too large for {NCORES} cores x {P} partitions"

    # ids >= G (if any) are dropped, matching segment_sum(num_segments=G)
    counts = np.bincount(b, minlength=NCORES * gpc)[: NCORES * gpc].astype(np.int64)
    starts = np.zeros(NCORES * gpc + 1, dtype=np.int64)
    np.cumsum(counts, out=starts[1:])
    t_max = int(counts.max()) if N else 1
    t_cap = max(B, math.ceil(t_max / B) * B)

    x_ext = np.vstack([x, np.zeros((1, F), dtype=np.float32)])  # row N = zeros
    col = np.arange(t_cap, dtype=np.int64)

    in_maps = []
    for k in range(NCORES):
        g0 = k * gpc
        cg = counts[g0 : g0 + gpc]
        sg = starts[g0 : g0 + gpc]
        valid = col[None, :] < cg[:, None]  # [gpc, t_cap]
        idx = np.where(valid, sg[:, None] + col[None, :], N)
        if gpc < P:  # pad partitions when graph count is not divisible by 8
            idx = np.vstack([idx, np.full((P - gpc, t_cap), N, dtype=np.int64)])

        feats = x_ext[idx]  # [P, t_cap, F] f32
        hi16 = np.ascontiguousarray(feats.astype(np.float16).reshape(P, t_cap * F))
        lo16 = np.ascontiguousarray(
            (feats - hi16.reshape(P, t_cap, F).astype(np.float32))
            .astype(np.float16)
            .reshape(P, t_cap * F)
        )

        inv = np.zeros((P, 1), dtype=np.float32)
        inv[:gpc, 0] = 1.0 / np.maximum(cg, 1)
        in_maps.append({"hi": hi16, "lo": lo16, "inv": inv})

    nc = _build(t_cap)
    try:
        res = run_bass_kernel_spmd(
            nc, in_maps, core_ids=list(range(NCORES)), trace=TRACE
        )
    except Exception:
        # transient device state (e.g. a previous run left a core wedged)
        # has been observed to clear on retry
        res = run_bass_kernel_spmd(
            nc, in_maps, core_ids=list(range(NCORES)), trace=TRACE
        )
    LAST_RESULTS = res

    out = np.concatenate([res.results[k]["out"] for k in range(NCORES)], axis=0)
    return out[:G]


# revision 28
# speedup vs baseline: 1.0608x; 1.0608x over previous
"""Trainium2 Bass kernel for batched global mean pooling (segment mean).

Computes, for N sorted nodes with 64 features and G graphs:
    out[g, f] = mean over nodes n with batch[n] == g of node_features[n, f]
(empty graphs -> zeros), distributed over 8 NeuronCores.

Strategy (graph sharding; no collectives):
  - Core k owns graphs [128k, 128(k+1)). batch is sorted, so each graph's
    nodes are a contiguous row range of node_features.
  - Host (inside kernel(), per call) lays out each core's nodes on a
    [128, T] grid: partition p gets only the nodes of local graph p,
    zero-padded to T = max graph size.
  - Features are split into fp16 (hi, lo) pairs so the PE runs at full
    rate (fp32 matmul is 4x slower); hi + lo recovers fp32 precision
    since the products accumulate into fp32 PSUM.
  - Device: each matmul is identity128.T @ slab for a [128, 7*64] fp16
    slab, accumulating into one [128, 448] f32 PSUM bank: partition =
    local graph. After all chunks: fold the 7 column blocks, multiply by
    host-provided 1/max(count, 1), DMA the [128, 64] result out.
  - Host concatenates the 8 per-core [128, 64] outputs.

The Bass program is compiled per call with the chunk count derived from
the actual input, so any node/graph distribution is handled.
"""

import math

import numpy as np

import concourse.mybir as mybir
import concourse.tile as tile
from concourse import bacc
from concourse.bass_utils import run_bass_kernel_spmd
from concourse.masks import make_identity

NCORES = 8
P = 128  # partitions = local graphs per core
F = 64  # features
B = 7  # tiles (node-rows) per matmul: N = 7*64 = 448 <= 512 f32 PSUM bank
TB = 63  # tiles per full DMA chunk (~1.03 MB per chunk)

# set by tests to capture a profile; harness path leaves these alone
TRACE = False
LAST_RESULTS = None


def _chunks(t_cap):
    """Split t_cap tiles into DMA chunks: small 21-tile chunks at the START
    (so the first chunk lands quickly and the PE starts early instead of
    trailing the stream by the whole prefetch depth) and at the END (short PE
    tail after the final DMA); full 63-tile chunks in between."""
    out = []
    t = 0
    taper = TB if t_cap > 8 * TB else 0
    while t < t_cap:
        in_taper = t < taper or t_cap - t <= taper
        n = min(3 * B if in_taper else TB, t_cap - t)
        out.append((t, n))
        t += n
    return out


def _build(t_cap):
    nc = bacc.Bacc("TRN2", target_bir_lowering=False, debug=False, num_devices=NCORES)
    # hi and lo halves of each chunk are packed back-to-back in one tensor so
    # every chunk is a single DMA: each DMA costs a semaphore, and the Tile
    # kernel-tail resets every used semaphore serially (~0.13 us each)
    hl = nc.dram_tensor(
        "hl", [P, 2 * t_cap * F], mybir.dt.float16, kind="ExternalInput"
    ).ap()
    inv = nc.dram_tensor("inv", [P, 1], mybir.dt.float32, kind="ExternalInput").ap()
    out = nc.dram_tensor("out", [P, F], mybir.dt.float32, kind="ExternalOutput").ap()

    chunks = _chunks(t_cap)
    n_mm = 2 * (t_cap // B)
    with tile.TileContext(nc) as tc:
        with (
            tc.tile_pool(name="consts", bufs=1) as consts,
            tc.tile_pool(name="io", bufs=5) as io,
            tc.tile_pool(name="ep", bufs=1) as ep,
            tc.tile_pool(name="acc", bufs=1, space="PSUM") as accp,
        ):
            # build the identity on-device (GpSimd) so the weight preload has
            # no DMA dependency -- an identity DMA would queue behind the
            # first big chunk DMAs and stall the PE ~14 us at kernel start
            ident_sb = consts.tile([P, P], mybir.dt.float16)
            make_identity(nc, ident_sb[:])

            # load the identity into the PE array once; every matmul below
            # reuses it (ldweights=False) instead of reloading 128 columns
            # per matmul (~100 ns each, ~60 us of PE time at ~600 matmuls)
            ldw = nc.tensor.ldweights(ident_sb[:])

            psum = accp.tile([P, B * F], mybir.dt.float32)
            mm = 0
            for ci, (t0, nt) in enumerate(chunks):
                hl_t = io.tile([P, 2 * TB * F], mybir.dt.float16, tag="hl")
                # alternate the two HWDGE rings (SP / ACT engines)
                eng = nc.sync if ci % 2 == 0 else nc.scalar
                eng.dma_start(
                    hl_t[:, : 2 * nt * F], hl[:, 2 * t0 * F : 2 * (t0 + nt) * F]
                )
                for b in range(2 * (nt // B)):
                    inst = nc.tensor.matmul(
                        psum[:],
                        ident_sb[:],
                        hl_t[:, b * B * F : (b + 1) * B * F],
                        start=(mm == 0),
                        stop=(mm == n_mm - 1),
                    )
                    inst.ins.ldweights = False
                    if mm == 0:
                        tile.add_dep_helper(
                            inst.ins,
                            ldw.ins,
                            sync=False,
                            reason="identity weights preloaded once",
                        )
                    mm += 1
            assert mm == n_mm

            # emitted after the chunk loop so this tiny transfer doesn't
            # head-of-line block the first chunk on the sync ring
            inv_sb = consts.tile([P, 1], mybir.dt.float32)
            nc.sync.dma_start(inv_sb[:], inv[:])

            # fold the B column blocks: s = sum_b psum[:, b*64:(b+1)*64]
            # (DVE may read at most one PSUM operand per instruction)
            s = ep.tile([P, F], mybir.dt.float32)
            nc.vector.tensor_copy(s[:], psum[:, 0:F])
            for b in range(1, B):
                nc.vector.tensor_add(s[:], s[:], psum[:, b * F : (b + 1) * F])

            res = ep.tile([P, F], mybir.dt.float32)
            nc.vector.tensor_scalar_mul(res[:], s[:], inv_sb[:])
            nc.sync.dma_start(out[:], res[:])

    nc.compile()
    # bacc materializes one Ldweights per Matmult even with ldweights=False;
    # they all reload the same identity (~100 ns of PE time each). Drop the
    # redundant ones — keep any that carry semaphore waits/updates (those
    # park sync state), including the explicit preload which waits on the
    # identity build.
    for fn in nc.m.functions:
        for blk in fn.blocks:
            keep = [
                inst
                for inst in blk.instructions
                if not (
                    isinstance(inst, mybir.InstLdweights)
                    and (
                        inst.sync_info is None
                        or (
                            len(inst.sync_info.on_wait) == 0
                            and len(inst.sync_info.on_update) == 0
                        )
                    )
                )
            ]
            if len(keep) != len(blk.instructions):
                blk.instructions = keep
    return nc


def kernel(node_features, batch, num_graphs):
    global LAST_RESULTS
    x = np.asarray(node_features, dtype=np.float32)
    b = np.asarray(batch, dtype=np.int64).ravel()
    G = int(num_graphs)
    N = x.shape[0]
    assert x.shape[1] == F, f"expected {F} features, got {x.shape[1]}"

    if not np.all(b[1:] >= b[:-1]):  # defensive: layout relies on sorted batch
        order = np.argsort(b, kind="stable")
        b = b[order]
        x = x[order]

    gpc = math.ceil(G / NCORES)  # local graphs per core
    assert gpc <= P, f"num_graphs {G} too large for {NCORES} cores x {P} partitions"

    # ids >= G (if any) are dropped, matching segment_sum(num_segments=G)
    counts = np.bincount(b, minlength=NCORES * gpc)[: NCORES * gpc].astype(np.int64)
    starts = np.zeros(NCORES * gpc + 1, dtype=np.int64)
    np.cumsum(counts, out=starts[1:])
    t_max = int(counts.max()) if N else 1
    t_cap = max(B, math.ceil(t_max / B) * B)

    x_ext = np.vstack([x, np.zeros((1, F), dtype=np.float32)])  # row N = zeros
    col = np.arange(t_cap, dtype=np.int64)
    chunk_list = _chunks(t_cap)

    in_maps = []
    for k in range(NCORES):
        g0 = k * gpc
        cg = counts[g0 : g0 + gpc]
        sg = starts[g0 : g0 + gpc]
        valid = col[None, :] < cg[:, None]  # [gpc, t_cap]
        idx = np.where(valid, sg[:, None] + col[None, :], N)
        if gpc < P:  # pad partitions when graph count is not divisible by 8
            idx = np.vstack([idx, np.full((P - gpc, t_cap), N, dtype=np.int64)])

        feats = x_ext[idx]  # [P, t_cap, F] f32
        hi16 = feats.astype(np.float16).reshape(P, t_cap * F)
        lo16 = (
            (feats - hi16.reshape(P, t_cap, F).astype(np.float32))
            .astype(np.float16)
            .reshape(P, t_cap * F)
        )
        # pack [hi-chunk | lo-chunk] back-to-back per chunk (see _build)
        hl = np.empty((P, 2 * t_cap * F), dtype=np.float16)
        for t0, nt in chunk_list:
            hl[:, 2 * t0 * F : (2 * t0 + nt) * F] = hi16[:, t0 * F : (t0 + nt) * F]
            hl[:, (2 * t0 + nt) * F : 2 * (t0 + nt) * F] = lo16[:, t0 * F : (t0 + nt) * F]

        inv = np.zeros((P, 1), dtype=np.float32)
        inv[:gpc, 0] = 1.0 / np.maximum(cg, 1)
        in_maps.append({"hl": hl, "inv": inv})

    nc = _build(t_cap)
    try:
        res = run_bass_kernel_spmd(
            nc, in_maps, core_ids=list(range(NCORES)), trace=TRACE
        )
    except Exception:
        # transient device state (e.g. a previous run left a core wedged)
        # has been observed to clear on retry
        res = run_bass_kernel_spmd(
            nc, in_maps, core_ids=list(range(NCORES)), trace=TRACE
        )
    LAST_RESULTS = res

    out = np.concatenate([res.results[k]["out"] for k in range(NCORES)], axis=0)
    return out[:G]
